# revision 1
# baseline (speedup 1.0000x reference)
"""Trainium2 Bass kernel for nn_MultiHeadAttention_31542239822105.

Math (faithful to reference, incl. softmax over the QUERY axis):
  q = einsum('bsd,hde->bhse', x, Wq) + bq ; same k, v
  scores = q @ k^T * 1/sqrt(DH)          [B,H,Sq,Sk]
  probs  = softmax(scores, axis=2)       # over q (query axis!)
  ctx    = einsum('bhqk,bhke->bhqe', probs, v)
  out    = ctx.reshape(B,S,D) @ Wo + bo

Sharding: data-parallel over batch, 8 cores x 8 batch items. No collectives.

Per-core layout strategy (all matmul contraction dims land on partitions):
  - x is pre-transposed on the HOST to xT [D, tokens] so no on-chip transposes.
  - Q^T,K^T come out of the projection f-major ([feat, token]) with W as the
    stationary operand; V comes out token-major with xT as stationary.
  - scoresT[k,q] = K^T.T @ Q^T per head -> softmax over q is a FREE-axis
    reduction; exp+sum fused into the PSUM eviction on ScalarE (accum_out).
  - 1/denominator is folded into V rows (cheap: S*DH vs S*S elements).
  - ctxT[f,q] accumulates per head pair into one PSUM tile; output projection
    uses ctxT chunks as stationary -> token-major result, direct DMA out.
  - 1/sqrt(DH) folded into Wq/bq on the host.
"""

import sys

if "/opt/trn_rl_repo" not in sys.path:
    sys.path.insert(0, "/opt/trn_rl_repo")

import numpy as np
import ml_dtypes

import concourse.bass as bass
import concourse.mybir as mybir
import concourse.tile as tile_mod
from concourse.vector_clock import ScopedClock
from concourse.bass_utils import run_bass_kernel_spmd

# ---------------------------------------------------------------- constants
B, S, D, H = 64, 577, 768, 12
DH = D // H          # 64
NCORES = 8
BC = B // NCORES     # 8 batch items per core
DC = D // 128        # 6 d-chunks
FC = D // 128        # 6 f-chunks per projection matrix
M_QK = 2 * FC        # 12 combined Q+K f-chunks
TT = (S + 127) // 128  # 5 token tiles (128,128,128,128,65)
S0 = 512             # PSUM-bank-sized free-dim split: 577 = 512 + 65
S1 = S - S0

BF16 = mybir.dt.bfloat16
F32 = mybir.dt.float32
nbf = ml_dtypes.bfloat16

_TILE_PATCHED = False
_CUR_NC = [None]


def _patch_tile_drain():
    """The walrus build here rejects >1 sync-wait per instruction
    ("Too many sync wait commands"). Two patches:
    1. post-legalize pass that moves extra waits onto single-wait nops
       inserted just before the offending instruction (same engine);
    2. the final SP Drain (emitted after legalize) gets the same split.
    """
    global _TILE_PATCHED
    if _TILE_PATCHED:
        return
    _TILE_PATCHED = True

    _orig_postorder = tile_mod.postorder_instruction_blocks

    def _split_multi_waits(ordered, nc):
        for bbname, insts in ordered.items():
            out = []
            n_split = 0
            for inst in insts:
                si = inst.sync_info
                if si is not None and len(si.on_wait) > 1:
                    waits = list(si.on_wait)
                    for w in waits[:-1]:
                        nop = mybir.InstNoOp(
                            name=nc.get_next_instruction_name(),
                            ins=[],
                            outs=[],
                            bass_is_fusable=False,
                        )
                        nop.engine = inst.engine
                        nop.sync_info = mybir.SyncInfo(on_wait=[w], on_update=[])
                        nc.register_instruction(nop, overwrite=True)
                        out.append(nop)
                        n_split += 1
                    inst.sync_info = mybir.SyncInfo(
                        on_wait=[waits[-1]], on_update=list(si.on_update)
                    )
                out.append(inst)
            ordered[bbname] = out
        return ordered

    def postorder_and_split(ordered, start_bb, postordered):
        # Runs post-sem-assignment, right before lowering: the only spot
        # where the final per-instruction waits are visible and editable.
        nc = _CUR_NC[0]
        _split_multi_waits(ordered, nc)
        return _orig_postorder(ordered, start_bb, postordered)

    tile_mod.postorder_instruction_blocks = postorder_and_split

    def _drain_and_barrier_split(self, tick_clock, wait_clock):
        nc = self.nc
        drain_inst = nc.sync.drain()
        wait_clock.add_sem_waits(
            drain_inst.ins, ScopedClock({None: tick_clock.global_clock})
        )
        si = drain_inst.ins.sync_info
        waits = list(si.on_wait)
        if len(waits) > 1:
            drain_inst.ins.sync_info = mybir.SyncInfo(
                on_wait=[waits[0]], on_update=list(si.on_update)
            )
            for w in waits[1:]:
                nop = nc.sync.nop(nofuse=True)
                nop.ins.sync_info = mybir.SyncInfo(on_wait=[w], on_update=[])
        nc.all_engine_barrier()
        assert self.sems is not None
        popped = nc._tile_sem_poison_stack.pop()
        assert popped is self._sem_poison
        nc.clear_and_free_semaphores(list(self.sems.allocated().values()))
        nc.all_engine_barrier()

    tile_mod.TileContext._drain_and_barrier = _drain_and_barrier_split


# ---------------------------------------------------------------- builder
def build_bass(bc=BC):
    """Emit the per-core kernel for `bc` batch items. Returns nc."""
    _patch_tile_drain()
    nc = bass.Bass()
    _CUR_NC[0] = nc

    xt_d = nc.declare_dram_parameter("xt", [DC, 128, bc, S], BF16, isOutput=False)
    wqk_d = nc.declare_dram_parameter("wqk", [128, M_QK, DC, 128], BF16, isOutput=False)
    wv_d = nc.declare_dram_parameter("wv", [128, DC, D], BF16, isOutput=False)
    wo_d = nc.declare_dram_parameter("wo", [128, FC, D], BF16, isOutput=False)
    bqk_d = nc.declare_dram_parameter("bqk", [128, M_QK], F32, isOutput=False)
    bvbc_d = nc.declare_dram_parameter("bvbc", [128, D], F32, isOutput=False)
    bobc_d = nc.declare_dram_parameter("bobc", [128, D], F32, isOutput=False)
    out_d = nc.declare_dram_parameter("out", [bc, S, D], F32, isOutput=True)

    AF = mybir.ActivationFunctionType

    with tile_mod.TileContext(nc) as tc:
        with (
            tc.tile_pool(name="singles", bufs=1) as singles,
            tc.tile_pool(name="xt", bufs=2) as xpool,
            tc.tile_pool(name="qk", bufs=2) as qkpool,
            tc.tile_pool(name="ktz", bufs=2) as kzpool,
            tc.tile_pool(name="v", bufs=2) as vpool,
            tc.tile_pool(name="probs", bufs=4) as ppool,
            tc.tile_pool(name="den", bufs=4) as dpool,
            tc.tile_pool(name="vs", bufs=4) as vspool,
            tc.tile_pool(name="ctx", bufs=2) as cpool,
            tc.tile_pool(name="ot", bufs=3) as opool,
            tc.tile_pool(name="psum", bufs=4, space="PSUM") as psum,
        ):
            # -------- resident weights / biases
            wqk = singles.tile([128, M_QK, DC, 128], BF16)
            nc.sync.dma_start(out=wqk, in_=wqk_d[:])
            wv = singles.tile([128, DC, D], BF16)
            nc.sync.dma_start(out=wv, in_=wv_d[:])
            wo = singles.tile([128, FC, D], BF16)
            nc.sync.dma_start(out=wo, in_=wo_d[:])
            bqk = singles.tile([128, M_QK], F32)
            nc.sync.dma_start(out=bqk, in_=bqk_d[:])
            bvbc = singles.tile([128, D], F32)
            nc.sync.dma_start(out=bvbc, in_=bvbc_d[:])
            bobc = singles.tile([128, D], F32)
            nc.sync.dma_start(out=bobc, in_=bobc_d[:])

            for b in range(bc):
                # -------- load xT_b [128, DC, S]
                xt = xpool.tile([128, DC, S], BF16, tag="xt")
                for dc in range(DC):
                    nc.sync.dma_start(out=xt[:, dc, :], in_=xt_d[dc, :, b, :])

                # -------- Q/K projections
                # Q -> qk [128, FC, S] (f-major, head pair per chunk)
                # K -> ktz [128, FC, 2, S]: zero-PADDED per head so the
                # scores lhsT is a full 128-partition operand (half-shape
                # matmuls run ~2x slow + drop the PE out of its fast clock).
                qk = qkpool.tile([128, FC, S], BF16, tag="qk")
                ktz = kzpool.tile([128, FC, 2, S], BF16, tag="ktz")
                for m in range(M_QK):
                    ps = psum.tile([128, D], F32, tag="ps")
                    for dc in range(DC):
                        st, sp = dc == 0, dc == DC - 1
                        nc.tensor.matmul(
                            ps[:, 0:S0], lhsT=wqk[:, m, dc, :], rhs=xt[:, dc, 0:S0],
                            start=st, stop=sp)
                        nc.tensor.matmul(
                            ps[:, S0:S], lhsT=wqk[:, m, dc, :], rhs=xt[:, dc, S0:S],
                            start=st, stop=sp)
                    if m < FC:
                        # evict Q + per-partition bias + cast to bf16
                        nc.scalar.activation(
                            qk[:, m, :], ps[:, 0:S], AF.Identity,
                            bias=bqk[:, m : m + 1], scale=1.0)
                    else:
                        mk = m - FC
                        nc.vector.memset(ktz[64:128, mk, 0, :], 0.0)
                        nc.vector.memset(ktz[0:64, mk, 1, :], 0.0)
                        nc.vector.tensor_scalar_add(
                            ktz[0:64, mk, 0, :], ps[0:64, 0:S],
                            bqk[0:64, m : m + 1])
                        nc.vector.tensor_scalar_add(
                            ktz[64:128, mk, 1, :], ps[64:128, 0:S],
                            bqk[64:128, m : m + 1])

                # -------- V projection -> v [128, TT, D] (token-major)
                v = vpool.tile([128, TT, D], BF16, tag="v")
                for tt in range(TT):
                    tsz = min(128, S - tt * 128)
                    t0 = tt * 128
                    ps = psum.tile([128, D], F32, tag="ps")
                    for dc in range(DC):
                        st, sp = dc == 0, dc == DC - 1
                        nc.tensor.matmul(
                            ps[:tsz, 0:S0], lhsT=xt[:, dc, t0 : t0 + tsz],
                            rhs=wv[:, dc, 0:S0], start=st, stop=sp)
                        nc.tensor.matmul(
                            ps[:tsz, S0:D], lhsT=xt[:, dc, t0 : t0 + tsz],
                            rhs=wv[:, dc, S0:D], start=st, stop=sp)
                    nc.vector.tensor_add(v[:tsz, tt, :], ps[:tsz, 0:D], bvbc[:tsz])

                # -------- attention per head -> ctxT [128, FC, S] (f-major)
                ctxT = cpool.tile([128, FC, S], BF16, tag="ctx")
                ps_c = None
                # Head loop is SOFTWARE-PIPELINED TWO heads deep, with the
                # ctx matmuls of head h-2 interleaved between the scores
                # matmul pairs of head h at kc granularity: the in-order PE
                # stream then always has independent work while ScalarE
                # drains the serial exp/denominator chain (827ns/tile vs
                # ~540ns PE fill rate); otherwise the PE micro-stalls on
                # PSUM slots and drops out of its fast clock state.
                hstate = {}  # h -> (probs, vsz)
                ps_c = None

                def emit_pipelined(h_s, h_c):
                    nonlocal ps_c
                    if h_s is not None:
                        po = (h_s % 2) * 64
                        m = h_s // 2
                        # rhs: full 128-partition Q chunk (both heads); the
                        # zero half of ktz masks the other head's rows.
                        qt2 = qk[:, m, :]
                        probs = ppool.tile([128, TT, S], BF16, tag="probs")
                        den = dpool.tile([128, TT], F32, tag="den")
                        nc.vector.memset(den, 1.0)
                    if h_c is not None:
                        probs_c, vsz_c = hstate.pop(h_c)
                        mc = h_c // 2
                        if h_c % 2 == 0:
                            ps_c = psum.tile([128, D], F32, tag="ps")
                    for kc in range(TT):
                        ksz = min(128, S - kc * 128)
                        k0 = kc * 128
                        if h_s is not None:
                            ps = psum.tile([128, D], F32, tag="ps")
                            nc.tensor.matmul(
                                ps[:ksz, 0:S0],
                                lhsT=ktz[:, m, h_s % 2, k0 : k0 + ksz],
                                rhs=qt2[:, 0:S0], start=True, stop=True)
                            nc.tensor.matmul(
                                ps[:ksz, S0:S],
                                lhsT=ktz[:, m, h_s % 2, k0 : k0 + ksz],
                                rhs=qt2[:, S0:S], start=True, stop=True)
                            # exp + row-sum (over q) fused in the eviction
                            nc.scalar.activation(
                                probs[:ksz, kc, :], ps[:ksz, 0:S], AF.Exp,
                                accum_out=den[:ksz, kc : kc + 1])
                        if h_c is not None:
                            st = (h_c % 2 == 0) and kc == 0
                            sp = (h_c % 2 == 1) and kc == TT - 1
                            nc.tensor.matmul(
                                ps_c[:, 0:S0], lhsT=vsz_c[:ksz, kc, :],
                                rhs=probs_c[:ksz, kc, 0:S0], start=st, stop=sp)
                            nc.tensor.matmul(
                                ps_c[:, S0:S], lhsT=vsz_c[:ksz, kc, :],
                                rhs=probs_c[:ksz, kc, S0:S], start=st, stop=sp)
                    if h_s is not None:
                        rd = dpool.tile([128, TT], F32, tag="rd")
                        nc.vector.reciprocal(rd, den)
                        # fold 1/denom into V rows; zero-pad the other
                        # head's column half so ctx lhsT is 128 cols wide
                        vsz = vspool.tile([128, TT, 128], BF16, tag="vsz")
                        nc.vector.memset(vsz[:, :, 64 - po : 128 - po], 0.0)
                        for kc in range(TT):
                            ksz = min(128, S - kc * 128)
                            nc.vector.tensor_scalar_mul(
                                vsz[:ksz, kc, po : po + 64],
                                v[:ksz, kc, h_s * DH : (h_s + 1) * DH],
                                rd[:ksz, kc : kc + 1])
                        hstate[h_s] = (probs, vsz)
                    if h_c is not None and h_c % 2 == 1:
                        nc.vector.tensor_copy(ctxT[:, mc, :], ps_c[:, 0:S])

                for i in range(H + 2):
                    emit_pipelined(
                        i if i < H else None, i - 2 if i >= 2 else None)

                # -------- output projection + bias -> DRAM (token-major)
                for tt in range(TT):
                    tsz = min(128, S - tt * 128)
                    t0 = tt * 128
                    ps = psum.tile([128, D], F32, tag="ps")
                    for fc in range(FC):
                        st, sp = fc == 0, fc == FC - 1
                        nc.tensor.matmul(
                            ps[:tsz, 0:S0], lhsT=ctxT[:, fc, t0 : t0 + tsz],
                            rhs=wo[:, fc, 0:S0], start=st, stop=sp)
                        nc.tensor.matmul(
                            ps[:tsz, S0:D], lhsT=ctxT[:, fc, t0 : t0 + tsz],
                            rhs=wo[:, fc, S0:D], start=st, stop=sp)
                    ot = opool.tile([128, D], F32, tag="ot")
                    nc.vector.tensor_add(ot[:tsz], ps[:tsz, 0:D], bobc[:tsz])
                    nc.sync.dma_start(
                        out=out_d[b, t0 : t0 + tsz, :], in_=ot[:tsz])

    return nc


# ---------------------------------------------------------------- host prep
def _prep_shared(Wq, bq, Wk, bk, Wv, bv, Wo, bo):
    """Build the per-core-identical weight operands."""
    scale = np.float32(1.0 / np.sqrt(DH))
    wqf = (Wq.astype(np.float32) * scale).transpose(1, 0, 2).reshape(D, D)
    wkf = Wk.astype(np.float32).transpose(1, 0, 2).reshape(D, D)
    wvf = Wv.astype(np.float32).transpose(1, 0, 2).reshape(D, D)

    def chunk4(wf):  # [d, f] -> [di, m, dc, fi]
        return wf.reshape(DC, 128, FC, 128).transpose(1, 2, 0, 3)

    wqk = np.concatenate([chunk4(wqf), chunk4(wkf)], axis=1)  # [128, 12, 6, 128]
    wv3 = wvf.reshape(DC, 128, D).transpose(1, 0, 2)          # [128, 6, 768]
    wo3 = Wo.astype(np.float32).reshape(FC, 128, D).transpose(1, 0, 2)

    bqf = (bq.astype(np.float32) * scale).reshape(D)
    bkf = bk.astype(np.float32).reshape(D)
    bqk = np.concatenate(
        [bqf.reshape(FC, 128), bkf.reshape(FC, 128)], axis=0
    ).T.copy()                                                # [128, 12]
    bvbc = np.broadcast_to(bv.astype(np.float32).reshape(D), (128, D)).copy()
    bobc = np.broadcast_to(bo.astype(np.float32).reshape(D), (128, D)).copy()

    return {
        "wqk": np.ascontiguousarray(wqk).astype(nbf),
        "wv": np.ascontiguousarray(wv3).astype(nbf),
        "wo": np.ascontiguousarray(wo3).astype(nbf),
        "bqk": np.ascontiguousarray(bqk),
        "bvbc": bvbc,
        "bobc": bobc,
    }


_NC_CACHE = {}


def kernel(x, Wq, bq, Wk, bk, Wv, bv, Wo, bo):
    x = np.asarray(x, dtype=np.float32)
    shared = _prep_shared(
        np.asarray(Wq), np.asarray(bq), np.asarray(Wk), np.asarray(bk),
        np.asarray(Wv), np.asarray(bv), np.asarray(Wo), np.asarray(bo))

    in_maps = []
    for c in range(NCORES):
        xc = x[c * BC : (c + 1) * BC]                    # [BC, S, D]
        xt = xc.transpose(2, 0, 1)                       # [D, BC, S]
        xt = xt.reshape(DC, 128, BC, S).astype(nbf)
        m = dict(shared)
        m["xt"] = np.ascontiguousarray(xt)
        in_maps.append(m)

    if "nc" not in _NC_CACHE:
        _NC_CACHE["nc"] = build_bass()
    nc = _NC_CACHE["nc"]

    res = run_bass_kernel_spmd(nc, in_maps, core_ids=list(range(NCORES)))
    out = np.concatenate([res.results[c]["out"] for c in range(NCORES)], axis=0)
    return out.astype(np.float32)


if __name__ == "__main__":
    rng = np.random.default_rng(0)
    ins = {
        "x": rng.standard_normal((B, S, D), dtype=np.float32),
        "Wq": rng.standard_normal((H, D, DH), dtype=np.float32) * 0.02,
        "bq": np.zeros((H, DH), np.float32),
        "Wk": rng.standard_normal((H, D, DH), dtype=np.float32) * 0.02,
        "bk": np.zeros((H, DH), np.float32),
        "Wv": rng.standard_normal((H, D, DH), dtype=np.float32) * 0.02,
        "bv": np.zeros((H, DH), np.float32),
        "Wo": rng.standard_normal((D, D), dtype=np.float32) * 0.02,
        "bo": np.zeros((D,), np.float32),
    }
    o = kernel(**ins)
    print("out", o.shape, o.dtype, float(np.abs(o).max()))

